# revision 8
# baseline (speedup 1.0000x reference)
"""MiniCausalAttention on 8 NeuronCores (Trainium2, Bass/Tile).

Problem: x[4,2048,1024] fp32; q/k/v = x@w+b; causal softmax(q k^T/sqrt(D)) @ v.

Sharding: 8 cores = (batch b in 0..3) x (half h in 0..1). Core (b,h) handles
query tiles t' = 2t+h for t in 0..7 (interleaved 128-row tiles), so every
core sees the SAME set of causal key-extents nk(t) = 256*(t+1) -> one SPMD
program, perfectly balanced. K^T and V are computed for HALF the sequence
per core (own token half) and pair-AllGathered (cores 2b, 2b+1 share batch
b), halving the projection FLOPs vs replication.

All matmuls run in bf16 (full PE rate); PSUM accumulation is fp32; softmax
statistics fp32. Layouts: x^T/K^T/Q^T are [d_model partition, token free] so
scores S[q,k] = (Q^T)^T K^T come out query-major; V is token-major so
O = P V after a PE transpose of each 128x128 P tile. P stays unnormalized
through P@V; O is scaled by 1/rowsum at copy-out and bvb = 1 (x) bv added.
"""

import sys

if "/opt/trn_rl_repo" not in sys.path:
    sys.path.insert(0, "/opt/trn_rl_repo")

import numpy as np
import ml_dtypes

import concourse.bass as bass  # noqa: F401
import concourse.tile as tile
from concourse import bacc, mybir
from concourse.bass_utils import run_bass_kernel_spmd
from concourse.masks import make_identity

BF16 = mybir.dt.bfloat16
F32 = mybir.dt.float32
AF = mybir.ActivationFunctionType

B, L, D = 4, 2048, 1024
P = 128
HL = L // 2      # token half per core
NQT = 8          # q-tiles per core, 128 rows each
SCALE = 1.0 / 32.0   # 1/sqrt(D)
NEG = -1.0e30
PAIRS = [[0, 1], [2, 3], [4, 5], [6, 7]]

_CACHED = {}


def build_nc():
    nc = bacc.Bacc(None, target_bir_lowering=False)

    xth = nc.declare_dram_parameter("xth", [D, HL], BF16, isOutput=False)
    xtq = nc.declare_dram_parameter("xtq", [D, D], BF16, isOutput=False)
    wq = nc.declare_dram_parameter("wq", [D, D], BF16, isOutput=False)
    wk = nc.declare_dram_parameter("wk", [D, D], BF16, isOutput=False)
    wv = nc.declare_dram_parameter("wv", [D, D], BF16, isOutput=False)
    bqm = nc.declare_dram_parameter("bqm", [P, 8], F32, isOutput=False)
    bkm = nc.declare_dram_parameter("bkm", [P, 8], F32, isOutput=False)
    bvr = nc.declare_dram_parameter("bvr", [1, D], BF16, isOutput=False)
    mask = nc.declare_dram_parameter("mask", [P, 256], F32, isOutput=False)
    out = nc.declare_dram_parameter("out", [D, D], F32, isOutput=True)

    with tile.TileContext(nc) as tc:
        with tc.tile_pool(name="persist", bufs=1) as persist, \
             tc.tile_pool(name="dram", bufs=1, space="DRAM") as dram:
            kt_sb = persist.tile([P, 8, L], BF16)    # K^T: [d-part, dt, token]
            v_sb = persist.tile([P, 16, D], BF16)    # V: [tok-part, tt, d]
            qt_sb = persist.tile([P, 8, D], BF16)    # Q^T: [d-part, dt, qcol]
            xtq_sb = persist.tile([P, 8, D], BF16)
            wq_sb = persist.tile([P, 8, D], BF16)
            bqm_sb = persist.tile([P, 8], F32)
            bkm_sb = persist.tile([P, 8], F32)
            bvr_sb = persist.tile([1, D], BF16)
            mask_sb = persist.tile([P, 256], F32)
            ident = persist.tile([P, P], BF16)
            ones_sb = persist.tile([1, P], BF16)
            bvb_sb = persist.tile([P, D], F32)       # broadcast bias 1 (x) bv

            gin = dram.tile([2 * HL, D], BF16)               # [V-half; KT-half]
            gout = dram.tile([2, 2 * HL, D], BF16)

            nc.sync.dma_start(out=bqm_sb, in_=bqm[:, :])
            nc.sync.dma_start(out=bkm_sb, in_=bkm[:, :])
            nc.sync.dma_start(out=bvr_sb, in_=bvr[:, :])
            nc.sync.dma_start(out=mask_sb, in_=mask[:, :])
            make_identity(nc, ident)
            nc.vector.memset(ones_sb, 1.0)

            # bvb = 1 (x) bv, built once via a K=1 matmul
            with tc.tile_pool(name="psB", bufs=1, space="PSUM") as psB:
                for dc in range(2):
                    pb = psB.tile([P, 512], F32, tag="pb", name="pb")
                    nc.tensor.matmul(pb, ones_sb, bvr_sb[:, dc * 512:(dc + 1) * 512],
                                     start=True, stop=True)
                    nc.scalar.copy(bvb_sb[:, dc * 512:(dc + 1) * 512], pb)

            # ---------- Phase A: V/K^T for own token half, then AllGather ---
            with tc.tile_pool(name="a_in", bufs=1) as a_in, \
                 tc.tile_pool(name="bounce", bufs=2) as bounce:
                wv_sb = a_in.tile([P, 8, D], BF16)
                xt_c = [a_in.tile([P, 8, 512], BF16, tag=f"xtc{c}", name=f"xtc{c}")
                        for c in range(2)]
                wk_sb = a_in.tile([P, 8, D], BF16)
                # DMA order: first what A1's first tiles need, then the rest.
                for i in range(8):
                    nc.sync.dma_start(out=wv_sb[:, i, :512],
                                      in_=wv[i * P:(i + 1) * P, :512])
                for i in range(8):
                    nc.sync.dma_start(out=xt_c[0][:, i, :],
                                      in_=xth[i * P:(i + 1) * P, :512])
                for i in range(8):
                    nc.sync.dma_start(out=wv_sb[:, i, 512:],
                                      in_=wv[i * P:(i + 1) * P, 512:])
                for i in range(8):
                    nc.sync.dma_start(out=xt_c[1][:, i, :],
                                      in_=xth[i * P:(i + 1) * P, 512:])
                for i in range(8):
                    nc.sync.dma_start(out=wk_sb[:, i, :], in_=wk[i * P:(i + 1) * P, :])
                # prefetch A3 inputs (used much later; persistent slots)
                for i in range(8):
                    nc.sync.dma_start(out=xtq_sb[:, i, :], in_=xtq[i * P:(i + 1) * P, :])
                for i in range(8):
                    nc.sync.dma_start(out=wq_sb[:, i, :], in_=wq[i * P:(i + 1) * P, :])

                # A1: V-half: V[tok, d] = sum_ct xth[ct, tok].T @ wv[ct, d]
                with tc.tile_pool(name="psV", bufs=3, space="PSUM") as psV:
                    for tt in range(8):
                        xtile = xt_c[tt // 4]
                        toff = (tt % 4) * P
                        pv = [psV.tile([P, 512], F32, tag=f"pv{dc}", name=f"pv{dc}")
                              for dc in range(2)]
                        for ct in range(8):
                            for dc in range(2):
                                nc.tensor.matmul(
                                    pv[dc],
                                    xtile[:, ct, toff:toff + P],
                                    wv_sb[:, ct, dc * 512:(dc + 1) * 512],
                                    start=(ct == 0),
                                    stop=(ct == 7),
                                )
                        vh = bounce.tile([P, D], BF16, tag="vh")
                        for dc in range(2):
                            nc.scalar.copy(vh[:, dc * 512:(dc + 1) * 512], pv[dc])
                        nc.sync.dma_start(out=gin[tt * P:(tt + 1) * P, :], in_=vh)

                # A2: K^T-half: K^T[d, tok] = sum_ct wk[ct, d].T @ xth[ct, tok]
                with tc.tile_pool(name="psK", bufs=2, space="PSUM") as psK:
                    for dt in range(8):
                        pk = [psK.tile([P, 512], F32, tag=f"pk{c}", name=f"pk{c}")
                              for c in range(2)]
                        for ct in range(8):
                            for c in range(2):
                                nc.tensor.matmul(
                                    pk[c],
                                    wk_sb[:, ct, dt * P:(dt + 1) * P],
                                    xt_c[c][:, ct, :],
                                    start=(ct == 0),
                                    stop=(ct == 7),
                                )
                        kh = bounce.tile([P, HL], BF16, tag="kh")
                        for c in range(2):
                            nc.scalar.activation(
                                kh[:, c * 512:(c + 1) * 512], pk[c],
                                AF.Identity, bias=bkm_sb[:, dt:dt + 1],
                            )
                        nc.sync.dma_start(
                            out=gin[HL + dt * P:HL + (dt + 1) * P, :], in_=kh)

                nc.gpsimd.collective_compute(
                    "AllGather",
                    mybir.AluOpType.bypass,
                    replica_groups=PAIRS,
                    ins=[gin[:, :]],
                    outs=[gout[:, :, :]],
                )

                # A3: Q^T for this core's 1024 rows (overlaps the collective)
                with tc.tile_pool(name="psQ", bufs=2, space="PSUM") as psQ:
                    for dt in range(8):
                        pq = [psQ.tile([P, 512], F32, tag=f"pq{c}", name=f"pq{c}")
                              for c in range(2)]
                        for ct in range(8):
                            for c in range(2):
                                nc.tensor.matmul(
                                    pq[c],
                                    wq_sb[:, ct, dt * P:(dt + 1) * P],
                                    xtq_sb[:, ct, c * 512:(c + 1) * 512],
                                    start=(ct == 0),
                                    stop=(ct == 7),
                                )
                        for c in range(2):
                            nc.scalar.activation(
                                qt_sb[:, dt, c * 512:(c + 1) * 512], pq[c],
                                AF.Identity, bias=bqm_sb[:, dt:dt + 1],
                            )

                # gather-in: rebuild full-position V and K^T from both ranks
                for r in range(2):
                    for tt in range(8):
                        nc.sync.dma_start(
                            out=v_sb[:, r * 8 + tt, :],
                            in_=gout[r, tt * P:(tt + 1) * P, :])
                    for dt in range(8):
                        nc.sync.dma_start(
                            out=kt_sb[:, dt, r * HL:(r + 1) * HL],
                            in_=gout[r, HL + dt * P:HL + (dt + 1) * P, :])

            # ---------------- Phase B: attention per q-tile ----------------
            with tc.tile_pool(name="bwork", bufs=2) as bwork, \
                 tc.tile_pool(name="psS", bufs=4, space="PSUM") as psS, \
                 tc.tile_pool(name="psT", bufs=2, space="PSUM") as psT, \
                 tc.tile_pool(name="psO", bufs=1, space="PSUM") as psO:
                for t in range(NQT):
                    nk = 256 * (t + 1)
                    nkc = (nk + 511) // 512  # 512-chunks (last may be 256)

                    p_sb = bwork.tile([P, 2048], BF16, tag="p")
                    rsum = bwork.tile([P, 4], F32, tag="rsum")
                    po = [psO.tile([P, 512], F32, tag=f"po{dc}", name=f"po{dc}")
                          for dc in range(2)]

                    for c in range(nkc):
                        w = min(512, nk - c * 512)
                        ps = psS.tile([P, 512], F32, tag="s")
                        for dt in range(8):
                            nc.tensor.matmul(
                                ps[:, :w],
                                qt_sb[:, dt, t * P:(t + 1) * P],
                                kt_sb[:, dt, c * 512:c * 512 + w],
                                start=(dt == 0),
                                stop=(dt == 7),
                            )
                        if c == nkc - 1:
                            nc.vector.tensor_add(ps[:, w - 256:w],
                                                 ps[:, w - 256:w], mask_sb)
                        nc.scalar.activation(
                            p_sb[:, c * 512:c * 512 + w], ps[:, :w], AF.Exp,
                            scale=SCALE, accum_out=rsum[:, c:c + 1])
                        for kt in range(c * 4, c * 4 + w // P):
                            ptp = psT.tile([P, P], BF16, tag="ptp")
                            nc.tensor.transpose(
                                ptp, p_sb[:, kt * P:(kt + 1) * P], ident)
                            pt_sb = bwork.tile([P, P], BF16, tag="pt")
                            nc.vector.tensor_copy(pt_sb, ptp)
                            for dc in range(2):
                                nc.tensor.matmul(
                                    po[dc],
                                    pt_sb,
                                    v_sb[:, kt, dc * 512:(dc + 1) * 512],
                                    start=(kt == 0),
                                    stop=(kt == nk // P - 1),
                                )

                    rinv = bwork.tile([P, 1], F32, tag="rinv")
                    rtot = bwork.tile([P, 1], F32, tag="rtot")
                    nc.vector.reduce_sum(rtot, rsum[:, :nkc], axis=mybir.AxisListType.X)
                    nc.vector.reciprocal(rinv, rtot)
                    o_sb = bwork.tile([P, D], F32, tag="o")
                    for dc in range(2):
                        sl = slice(dc * 512, (dc + 1) * 512)
                        nc.scalar.activation(o_sb[:, sl], po[dc], AF.Copy,
                                             scale=rinv)
                        nc.vector.tensor_add(o_sb[:, sl], o_sb[:, sl],
                                             bvb_sb[:, sl])
                    nc.sync.dma_start(out=out[t * P:(t + 1) * P, :], in_=o_sb)

    nc.finalize()
    return nc


def _prep_inputs(x, wq, bq, wk, bk, wv, bv):
    bf = ml_dtypes.bfloat16
    wq_b = np.ascontiguousarray(wq, np.float32).astype(bf)
    wk_b = np.ascontiguousarray(wk, np.float32).astype(bf)
    wv_b = np.ascontiguousarray(wv, np.float32).astype(bf)
    bqm = np.ascontiguousarray(np.asarray(bq, np.float32).reshape(8, P).T)
    bkm = np.ascontiguousarray(np.asarray(bk, np.float32).reshape(8, P).T)
    bvr = np.asarray(bv, np.float32).reshape(1, D).astype(bf)

    i = np.arange(P)[:, None]
    j = np.arange(256)[None, :]
    masks = [np.where(j <= i + P * h, 0.0, NEG).astype(np.float32)
             for h in range(2)]

    in_maps = []
    for core in range(8):
        b, h = core // 2, core % 2
        xT = np.ascontiguousarray(np.asarray(x[b], np.float32).T).astype(bf)
        xthalf = np.ascontiguousarray(xT[:, h * HL:(h + 1) * HL])
        qcols = (np.arange(8)[:, None] * 2 + h) * P + np.arange(P)[None, :]
        xTq = np.ascontiguousarray(xT[:, qcols.ravel()])
        in_maps.append({
            "xth": xthalf, "xtq": xTq, "wq": wq_b, "wk": wk_b, "wv": wv_b,
            "bqm": bqm, "bkm": bkm, "bvr": bvr, "mask": masks[h],
        })
    return in_maps


def kernel(x, wq, bq, wk, bk, wv, bv, _trace=False, _trace_kwargs=None):
    if "nc" not in _CACHED:
        _CACHED["nc"] = build_nc()
    nc = _CACHED["nc"]
    in_maps = _prep_inputs(x, wq, bq, wk, bk, wv, bv)
    kw = {}
    if _trace:
        kw = dict(trace=True, **(_trace_kwargs or {}))
    res = run_bass_kernel_spmd(nc, in_maps, list(range(8)), **kw)
    out = np.empty((B, L, D), np.float32)
    for core in range(8):
        b, h = core // 2, core % 2
        o = np.asarray(res.results[core]["out"], np.float32)
        out[b].reshape(16, P, D)[h::2] = o.reshape(NQT, P, D)
    if _trace:
        _CACHED["last_results"] = res
    return out


# revision 10
# speedup vs baseline: 1.6730x; 1.6730x over previous
"""MiniCausalAttention on 8 NeuronCores (Trainium2, Bass/Tile).

Problem: x[4,2048,1024] fp32; q/k/v = x@w+b; causal softmax(q k^T/sqrt(D)) @ v.

Sharding: 8 cores = (batch b in 0..3) x (half h in 0..1). Core (b,h) handles
query tiles t' = 2t+h for t in 0..7 (interleaved 128-row tiles), so every
core sees the SAME set of causal key-extents nk(t) = 256*(t+1) -> one SPMD
program, perfectly balanced.

Projection reassociation (exact algebra, host-precomputed M = Wq Wk^T and
u = Wk bq):
  scores  S = q k^T = x_q M x^T + 1 (x) (x u)^T  (+ per-query terms that
          cancel in softmax and are dropped)
  output  O = P_norm v = [(P x) Wv] / rowsum + bv
so neither K nor V is ever materialized: phase A computes only
G^T = (x_q M)^T and the key-bias row c' = (x u)^T; the c' term is folded
into the S accumulation as a K=1 matmul of ones^T (x) c'.

All matmuls run in bf16 (full PE rate); PSUM accumulation fp32; softmax
statistics fp32. G^T/x^T are [d_model partition, token free] so S comes out
query-major; x rows are token-major for Z = P x after a PE transpose of each
128x128 P tile; Z is PE-transposed again for O = Z Wv.
"""

import sys

if "/opt/trn_rl_repo" not in sys.path:
    sys.path.insert(0, "/opt/trn_rl_repo")

import numpy as np
import ml_dtypes

import concourse.bass as bass  # noqa: F401
import concourse.tile as tile
from concourse import bacc, mybir
from concourse.bass_utils import run_bass_kernel_spmd
from concourse.masks import make_identity

BF16 = mybir.dt.bfloat16
F32 = mybir.dt.float32
AF = mybir.ActivationFunctionType

B, L, D = 4, 2048, 1024
P = 128
NQT = 8          # q-tiles per core, 128 rows each
SCALE = 1.0 / 32.0   # 1/sqrt(D)
NEG = -1.0e30

_CACHED = {}


def build_nc():
    nc = bacc.Bacc(None, target_bir_lowering=False)

    xt = nc.declare_dram_parameter("xt", [D, L], BF16, isOutput=False)    # x^T
    xr = nc.declare_dram_parameter("xr", [L, D], BF16, isOutput=False)    # x rows
    xtq = nc.declare_dram_parameter("xtq", [D, D], BF16, isOutput=False)  # q cols of x^T
    mm_w = nc.declare_dram_parameter("mm_w", [D, D], BF16, isOutput=False)  # Wq Wk^T
    wv = nc.declare_dram_parameter("wv", [D, D], BF16, isOutput=False)
    um = nc.declare_dram_parameter("um", [P, 8], BF16, isOutput=False)    # Wk bq
    bvr = nc.declare_dram_parameter("bvr", [1, D], BF16, isOutput=False)
    mask = nc.declare_dram_parameter("mask", [P, 256], F32, isOutput=False)
    out = nc.declare_dram_parameter("out", [D, D], F32, isOutput=True)

    with tile.TileContext(nc) as tc:
        with tc.tile_pool(name="persist", bufs=1) as persist:
            xt_sb = persist.tile([P, 8, L], BF16)    # x^T: [d-part, ct, token]
            xr_sb = persist.tile([P, 16, D], BF16)   # x: [tok-part, tt, d]
            gt_sb = persist.tile([P, 8, D], BF16)    # G^T: [d-part, dt, qcol]
            xtq_sb = persist.tile([P, 8, D], BF16)
            m_sb = persist.tile([P, 8, D], BF16)
            wv_sb = persist.tile([P, 8, D], BF16)
            um_sb = persist.tile([P, 8], BF16)
            cx_sb = persist.tile([1, L], BF16)       # c' = (x u)^T key-bias row
            bvr_sb = persist.tile([1, D], BF16)
            mask_sb = persist.tile([P, 256], F32)
            ident = persist.tile([P, P], BF16)
            ones_sb = persist.tile([1, P], BF16)
            bvb_sb = persist.tile([P, D], F32)       # broadcast bias 1 (x) bv

            nc.sync.dma_start(out=um_sb, in_=um[:, :])
            nc.sync.dma_start(out=bvr_sb, in_=bvr[:, :])
            nc.sync.dma_start(out=mask_sb, in_=mask[:, :])
            make_identity(nc, ident)
            nc.vector.memset(ones_sb, 1.0)

            # input streams, roughly in first-use order
            for i in range(8):
                nc.sync.dma_start(out=xt_sb[:, i, :512], in_=xt[i * P:(i + 1) * P, :512])
            for i in range(8):
                nc.sync.dma_start(out=xtq_sb[:, i, :], in_=xtq[i * P:(i + 1) * P, :])
            for i in range(8):
                nc.sync.dma_start(out=m_sb[:, i, :], in_=mm_w[i * P:(i + 1) * P, :])
            for c in range(1, 4):
                for i in range(8):
                    nc.sync.dma_start(out=xt_sb[:, i, c * 512:(c + 1) * 512],
                                      in_=xt[i * P:(i + 1) * P, c * 512:(c + 1) * 512])
            for tt in range(16):
                nc.sync.dma_start(out=xr_sb[:, tt, :],
                                  in_=xr[tt * P:(tt + 1) * P, :])
            for i in range(8):
                nc.sync.dma_start(out=wv_sb[:, i, :], in_=wv[i * P:(i + 1) * P, :])

            # bvb = 1 (x) bv, built once via a K=1 matmul
            with tc.tile_pool(name="psB", bufs=1, space="PSUM") as psB:
                for dc in range(2):
                    pb = psB.tile([P, 512], F32, tag="pb", name="pb")
                    nc.tensor.matmul(pb, ones_sb, bvr_sb[:, dc * 512:(dc + 1) * 512],
                                     start=True, stop=True)
                    nc.scalar.copy(bvb_sb[:, dc * 512:(dc + 1) * 512], pb)

            # ---------- Phase A: G^T = (x_q M)^T and c' = (x u)^T ----------
            # qc-outer so the first half of G is ready early for phase B.
            with tc.tile_pool(name="psG", bufs=2, space="PSUM") as psG:
                for qc in range(2):
                    for dt in range(8):
                        pg = psG.tile([P, 512], F32, tag="pg", name="pg")
                        for ct in range(8):
                            nc.tensor.matmul(
                                pg,
                                m_sb[:, ct, dt * P:(dt + 1) * P],
                                xtq_sb[:, ct, qc * 512:(qc + 1) * 512],
                                start=(ct == 0),
                                stop=(ct == 7),
                            )
                        nc.scalar.copy(gt_sb[:, dt, qc * 512:(qc + 1) * 512], pg)

                # c' row: for each 512-token chunk, sum_ct u[ct]^T @ x^T[ct]
                for c in range(4):
                    pc = psG.tile([1, 512], F32, tag="pc", name="pc")
                    for ct in range(8):
                        nc.tensor.matmul(
                            pc,
                            um_sb[:, ct:ct + 1],
                            xt_sb[:, ct, c * 512:(c + 1) * 512],
                            start=(ct == 0),
                            stop=(ct == 7),
                        )
                    nc.scalar.copy(cx_sb[:, c * 512:(c + 1) * 512], pc)

            # ---------------- Phase B: attention per q-tile ----------------
            # S chunks (+ c' fold-in) -> exp (+rowsum) -> P^T -> Z = P x
            # -> Z^T -> O = Z^T.T Wv, scaled by 1/rowsum, + bvb.
            with tc.tile_pool(name="bwork", bufs=2) as bwork, \
                 tc.tile_pool(name="psS", bufs=2, space="PSUM") as psS, \
                 tc.tile_pool(name="psT", bufs=2, space="PSUM") as psT, \
                 tc.tile_pool(name="psZ", bufs=1, space="PSUM") as psZ, \
                 tc.tile_pool(name="psO", bufs=1, space="PSUM") as psO:
                for t in range(NQT):
                    nk = 256 * (t + 1)
                    nkc = (nk + 511) // 512  # 512-chunks (last may be 256)

                    p_sb = bwork.tile([P, 2048], BF16, tag="p")
                    rsum = bwork.tile([P, 4], F32, tag="rsum")
                    pz = [psZ.tile([P, 512], F32, tag=f"pz{dc}", name=f"pz{dc}")
                          for dc in range(2)]

                    for c in range(nkc):
                        w = min(512, nk - c * 512)
                        ps = psS.tile([P, 512], F32, tag="s")
                        for dt in range(8):
                            nc.tensor.matmul(
                                ps[:, :w],
                                gt_sb[:, dt, t * P:(t + 1) * P],
                                xt_sb[:, dt, c * 512:c * 512 + w],
                                start=(dt == 0),
                                stop=False,
                            )
                        # key-bias c' via K=1 matmul of ones^T (x) cx
                        nc.tensor.matmul(
                            ps[:, :w],
                            ones_sb,
                            cx_sb[:, c * 512:c * 512 + w],
                            start=False,
                            stop=True,
                        )
                        if c == nkc - 1:
                            nc.vector.tensor_add(ps[:, w - 256:w],
                                                 ps[:, w - 256:w], mask_sb)
                        nc.scalar.activation(
                            p_sb[:, c * 512:c * 512 + w], ps[:, :w], AF.Exp,
                            scale=SCALE, accum_out=rsum[:, c:c + 1])
                        for kt in range(c * 4, c * 4 + w // P):
                            ptp = psT.tile([P, P], BF16, tag="ptp")
                            nc.tensor.transpose(
                                ptp, p_sb[:, kt * P:(kt + 1) * P], ident)
                            pt_sb = bwork.tile([P, P], BF16, tag="pt")
                            nc.vector.tensor_copy(pt_sb, ptp)
                            for dc in range(2):
                                nc.tensor.matmul(
                                    pz[dc],
                                    pt_sb,
                                    xr_sb[:, kt, dc * 512:(dc + 1) * 512],
                                    start=(kt == 0),
                                    stop=(kt == nk // P - 1),
                                )

                    # Z -> SBUF (bf16), then Z^T tiles, then O = Z Wv
                    z_sb = bwork.tile([P, D], BF16, tag="z")
                    for dc in range(2):
                        nc.scalar.copy(z_sb[:, dc * 512:(dc + 1) * 512], pz[dc])
                    po = [psO.tile([P, 512], F32, tag=f"po{dc}", name=f"po{dc}")
                          for dc in range(2)]
                    for cc in range(8):
                        ztp = psT.tile([P, P], BF16, tag="ptp", name="ztp")
                        nc.tensor.transpose(ztp, z_sb[:, cc * P:(cc + 1) * P], ident)
                        zt_sb = bwork.tile([P, P], BF16, tag="zt")
                        nc.vector.tensor_copy(zt_sb, ztp)
                        for dc in range(2):
                            nc.tensor.matmul(
                                po[dc],
                                zt_sb,
                                wv_sb[:, cc, dc * 512:(dc + 1) * 512],
                                start=(cc == 0),
                                stop=(cc == 7),
                            )

                    rinv = bwork.tile([P, 1], F32, tag="rinv")
                    rtot = bwork.tile([P, 1], F32, tag="rtot")
                    nc.vector.reduce_sum(rtot, rsum[:, :nkc], axis=mybir.AxisListType.X)
                    nc.vector.reciprocal(rinv, rtot)
                    o_sb = bwork.tile([P, D], F32, tag="o")
                    for dc in range(2):
                        sl = slice(dc * 512, (dc + 1) * 512)
                        nc.scalar.activation(o_sb[:, sl], po[dc], AF.Copy,
                                             scale=rinv)
                        nc.vector.tensor_add(o_sb[:, sl], o_sb[:, sl],
                                             bvb_sb[:, sl])
                    nc.sync.dma_start(out=out[t * P:(t + 1) * P, :], in_=o_sb)

    nc.finalize()
    return nc


def _prep_inputs(x, wq, bq, wk, bk, wv, bv):
    bf = ml_dtypes.bfloat16
    wq32 = np.asarray(wq, np.float32)
    wk32 = np.asarray(wk, np.float32)
    m_host = (wq32 @ wk32.T).astype(bf)                 # Wq Wk^T
    u_host = (wk32 @ np.asarray(bq, np.float32))        # Wk bq, [D]
    um = np.ascontiguousarray(u_host.reshape(8, P).T).astype(bf)
    wv_b = np.ascontiguousarray(wv, np.float32).astype(bf)
    bvr = np.asarray(bv, np.float32).reshape(1, D).astype(bf)

    i = np.arange(P)[:, None]
    j = np.arange(256)[None, :]
    masks = [np.where(j <= i + P * h, 0.0, NEG).astype(np.float32)
             for h in range(2)]

    in_maps = []
    for core in range(8):
        b, h = core // 2, core % 2
        xb = np.asarray(x[b], np.float32)
        xT = np.ascontiguousarray(xb.T).astype(bf)
        xR = xb.astype(bf)
        qcols = (np.arange(8)[:, None] * 2 + h) * P + np.arange(P)[None, :]
        xTq = np.ascontiguousarray(xT[:, qcols.ravel()])
        in_maps.append({
            "xt": xT, "xr": xR, "xtq": xTq, "mm_w": m_host, "wv": wv_b,
            "um": um, "bvr": bvr, "mask": masks[h],
        })
    return in_maps


def kernel(x, wq, bq, wk, bk, wv, bv, _trace=False, _trace_kwargs=None):
    if "nc" not in _CACHED:
        _CACHED["nc"] = build_nc()
    nc = _CACHED["nc"]
    in_maps = _prep_inputs(x, wq, bq, wk, bk, wv, bv)
    kw = {}
    if _trace:
        kw = dict(trace=True, **(_trace_kwargs or {}))
    res = run_bass_kernel_spmd(nc, in_maps, list(range(8)), **kw)
    out = np.empty((B, L, D), np.float32)
    for core in range(8):
        b, h = core // 2, core % 2
        o = np.asarray(res.results[core]["out"], np.float32)
        out[b].reshape(16, P, D)[h::2] = o.reshape(NQT, P, D)
    if _trace:
        _CACHED["last_results"] = res
    return out


# revision 11
# speedup vs baseline: 1.6962x; 1.0139x over previous
"""MiniCausalAttention on 8 NeuronCores (Trainium2, Bass/Tile).

Problem: x[4,2048,1024] fp32; q/k/v = x@w+b; causal softmax(q k^T/sqrt(D)) @ v.

Sharding: 8 cores = (batch b in 0..3) x (half h in 0..1). Core (b,h) handles
query tiles t' = 2t+h for t in 0..7 (interleaved 128-row tiles), so every
core sees the SAME set of causal key-extents nk(t) = 256*(t+1) -> one SPMD
program, perfectly balanced.

Projection reassociation (exact algebra, host-precomputed M = Wq Wk^T and
u = Wk bq):
  scores  S = q k^T = x_q M x^T + 1 (x) (x u)^T  (+ per-query terms that
          cancel in softmax and are dropped)
  output  O = P_norm v = [(P x) Wv] / rowsum + bv
so neither K nor V is ever materialized: phase A computes only
G^T = (x_q M)^T and the key-bias row c' = (x u)^T; the c' term is folded
into the S accumulation as a K=1 matmul of ones^T (x) c'.

All matmuls run in bf16 (full PE rate); PSUM accumulation fp32; softmax
statistics fp32. G^T/x^T are [d_model partition, token free] so S comes out
query-major; x rows are token-major for Z = P x after a PE transpose of each
128x128 P tile; Z is PE-transposed again for O = Z Wv.
"""

import sys

if "/opt/trn_rl_repo" not in sys.path:
    sys.path.insert(0, "/opt/trn_rl_repo")

import numpy as np
import ml_dtypes

import concourse.bass as bass  # noqa: F401
import concourse.tile as tile
from concourse import bacc, mybir
from concourse.bass_utils import run_bass_kernel_spmd
from concourse.masks import make_identity

BF16 = mybir.dt.bfloat16
F32 = mybir.dt.float32
AF = mybir.ActivationFunctionType

B, L, D = 4, 2048, 1024
P = 128
NQT = 8          # q-tiles per core, 128 rows each
SCALE = 1.0 / 32.0   # 1/sqrt(D)
NEG = -1.0e30

_CACHED = {}


def build_nc():
    nc = bacc.Bacc(None, target_bir_lowering=False)

    xt = nc.declare_dram_parameter("xt", [D, L], BF16, isOutput=False)    # x^T
    xr = nc.declare_dram_parameter("xr", [L, D], BF16, isOutput=False)    # x rows
    xtq = nc.declare_dram_parameter("xtq", [D, D], BF16, isOutput=False)  # q cols of x^T
    mm_w = nc.declare_dram_parameter("mm_w", [D, D], BF16, isOutput=False)  # Wq Wk^T
    wv = nc.declare_dram_parameter("wv", [D, D], BF16, isOutput=False)
    um = nc.declare_dram_parameter("um", [P, 8], BF16, isOutput=False)    # Wk bq
    bvr = nc.declare_dram_parameter("bvr", [1, D], BF16, isOutput=False)
    mask = nc.declare_dram_parameter("mask", [P, 256], F32, isOutput=False)
    out = nc.declare_dram_parameter("out", [D, D], F32, isOutput=True)

    with tile.TileContext(nc) as tc:
        with tc.tile_pool(name="persist", bufs=1) as persist:
            xt_sb = persist.tile([P, 8, L], BF16)    # x^T: [d-part, ct, token]
            xr_sb = persist.tile([P, 16, D], BF16)   # x: [tok-part, tt, d]
            gt_sb = persist.tile([P, 8, D], BF16)    # G^T: [d-part, dt, qcol]
            xtq_sb = persist.tile([P, 8, D], BF16)
            m_sb = persist.tile([P, 8, D], BF16)
            wv_sb = persist.tile([P, 8, D], BF16)
            um_sb = persist.tile([P, 8], BF16)
            cx_sb = persist.tile([1, L], BF16)       # c' = (x u)^T key-bias row
            bvr_sb = persist.tile([1, D], BF16)
            mask_sb = persist.tile([P, 256], F32)
            ident = persist.tile([P, P], BF16)
            ones_sb = persist.tile([1, P], BF16)
            bvb_sb = persist.tile([P, D], F32)       # broadcast bias 1 (x) bv

            nc.sync.dma_start(out=um_sb, in_=um[:, :])
            nc.sync.dma_start(out=bvr_sb, in_=bvr[:, :])
            nc.sync.dma_start(out=mask_sb, in_=mask[:, :])
            make_identity(nc, ident)
            nc.vector.memset(ones_sb, 1.0)

            # input streams, roughly in first-use order
            for i in range(8):
                nc.sync.dma_start(out=xt_sb[:, i, :512], in_=xt[i * P:(i + 1) * P, :512])
            for i in range(8):
                nc.sync.dma_start(out=m_sb[:, i, :], in_=mm_w[i * P:(i + 1) * P, :])
            for i in range(8):
                nc.sync.dma_start(out=xtq_sb[:, i, :], in_=xtq[i * P:(i + 1) * P, :])
            for c in range(1, 4):
                for i in range(8):
                    nc.sync.dma_start(out=xt_sb[:, i, c * 512:(c + 1) * 512],
                                      in_=xt[i * P:(i + 1) * P, c * 512:(c + 1) * 512])
            for tt in range(16):
                nc.sync.dma_start(out=xr_sb[:, tt, :],
                                  in_=xr[tt * P:(tt + 1) * P, :])
            for i in range(8):
                nc.sync.dma_start(out=wv_sb[:, i, :], in_=wv[i * P:(i + 1) * P, :])

            # Two PSUM pools spanning phases A and B (8 banks total):
            # psA: tags s (x2) + ptp/pc (x2); psC: pz0, pz1, po0, po1.
            with tc.tile_pool(name="bwork", bufs=2) as bwork, \
                 tc.tile_pool(name="psA", bufs=2, space="PSUM") as psS, \
                 tc.tile_pool(name="psC", bufs=1, space="PSUM") as psZ:
                psT = psS   # transposes + c' share the psA pool (tag ptp)
                psO = psZ

                # bvb = 1 (x) bv, built once via a K=1 matmul
                for dc in range(2):
                    pb = psZ.tile([P, 512], F32, tag=f"po{dc}", name=f"pb{dc}")
                    nc.tensor.matmul(pb, ones_sb, bvr_sb[:, dc * 512:(dc + 1) * 512],
                                     start=True, stop=True)
                    nc.scalar.copy(bvb_sb[:, dc * 512:(dc + 1) * 512], pb)

                # ---------- Phase A: c' = (x u)^T and G^T = (x_q M)^T ------
                # c' chunk 0 only needs the first 1 MB of x^T: earliest PE
                # work. G runs qc-outer so B's first tiles unblock early.
                def cprime_chunk(c):
                    pc = psS.tile([1, 512], F32, tag="ptp", name=f"pc{c}")
                    for ct in range(8):
                        nc.tensor.matmul(
                            pc,
                            um_sb[:, ct:ct + 1],
                            xt_sb[:, ct, c * 512:(c + 1) * 512],
                            start=(ct == 0),
                            stop=(ct == 7),
                        )
                    nc.scalar.copy(cx_sb[:, c * 512:(c + 1) * 512], pc)

                cprime_chunk(0)
                for qc in range(2):
                    for dt in range(8):
                        pg = psS.tile([P, 512], F32, tag="s", name="pg")
                        for ct in range(8):
                            nc.tensor.matmul(
                                pg,
                                m_sb[:, ct, dt * P:(dt + 1) * P],
                                xtq_sb[:, ct, qc * 512:(qc + 1) * 512],
                                start=(ct == 0),
                                stop=(ct == 7),
                            )
                        nc.scalar.copy(gt_sb[:, dt, qc * 512:(qc + 1) * 512], pg)
                    if qc == 0:
                        for c in range(1, 4):
                            cprime_chunk(c)

                # ------------- Phase B: attention per q-tile ---------------
                # S chunks (+ c' fold-in) -> exp (+rowsum) -> P^T -> Z = P x
                # -> Z^T -> O = Z^T.T Wv, scaled by 1/rowsum, + bvb.
                for t in range(NQT):
                    nk = 256 * (t + 1)
                    nkc = (nk + 511) // 512  # 512-chunks (last may be 256)

                    p_sb = bwork.tile([P, 2048], BF16, tag="p")
                    rsum = bwork.tile([P, 4], F32, tag="rsum")
                    pz = [psZ.tile([P, 512], F32, tag=f"pz{dc}", name=f"pz{dc}")
                          for dc in range(2)]

                    for c in range(nkc):
                        w = min(512, nk - c * 512)
                        ps = psS.tile([P, 512], F32, tag="s")
                        for dt in range(8):
                            nc.tensor.matmul(
                                ps[:, :w],
                                gt_sb[:, dt, t * P:(t + 1) * P],
                                xt_sb[:, dt, c * 512:c * 512 + w],
                                start=(dt == 0),
                                stop=False,
                            )
                        # key-bias c' via K=1 matmul of ones^T (x) cx
                        nc.tensor.matmul(
                            ps[:, :w],
                            ones_sb,
                            cx_sb[:, c * 512:c * 512 + w],
                            start=False,
                            stop=True,
                        )
                        if c == nkc - 1:
                            nc.vector.tensor_add(ps[:, w - 256:w],
                                                 ps[:, w - 256:w], mask_sb)
                        nc.scalar.activation(
                            p_sb[:, c * 512:c * 512 + w], ps[:, :w], AF.Exp,
                            scale=SCALE, accum_out=rsum[:, c:c + 1])
                        for kt in range(c * 4, c * 4 + w // P):
                            ptp = psT.tile([P, P], BF16, tag="ptp")
                            nc.tensor.transpose(
                                ptp, p_sb[:, kt * P:(kt + 1) * P], ident)
                            pt_sb = bwork.tile([P, P], BF16, tag="pt")
                            nc.vector.tensor_copy(pt_sb, ptp)
                            for dc in range(2):
                                nc.tensor.matmul(
                                    pz[dc],
                                    pt_sb,
                                    xr_sb[:, kt, dc * 512:(dc + 1) * 512],
                                    start=(kt == 0),
                                    stop=(kt == nk // P - 1),
                                )

                    # Z -> SBUF (bf16), then Z^T tiles, then O = Z Wv
                    z_sb = bwork.tile([P, D], BF16, tag="z")
                    for dc in range(2):
                        nc.scalar.copy(z_sb[:, dc * 512:(dc + 1) * 512], pz[dc])
                    po = [psO.tile([P, 512], F32, tag=f"po{dc}", name=f"po{dc}")
                          for dc in range(2)]
                    for cc in range(8):
                        ztp = psT.tile([P, P], BF16, tag="ptp", name="ztp")
                        nc.tensor.transpose(ztp, z_sb[:, cc * P:(cc + 1) * P], ident)
                        zt_sb = bwork.tile([P, P], BF16, tag="zt")
                        nc.vector.tensor_copy(zt_sb, ztp)
                        for dc in range(2):
                            nc.tensor.matmul(
                                po[dc],
                                zt_sb,
                                wv_sb[:, cc, dc * 512:(dc + 1) * 512],
                                start=(cc == 0),
                                stop=(cc == 7),
                            )

                    rinv = bwork.tile([P, 1], F32, tag="rinv")
                    rtot = bwork.tile([P, 1], F32, tag="rtot")
                    nc.vector.reduce_sum(rtot, rsum[:, :nkc], axis=mybir.AxisListType.X)
                    nc.vector.reciprocal(rinv, rtot)
                    o_sb = bwork.tile([P, D], F32, tag="o")
                    for dc in range(2):
                        sl = slice(dc * 512, (dc + 1) * 512)
                        nc.scalar.activation(o_sb[:, sl], po[dc], AF.Copy,
                                             scale=rinv)
                        nc.vector.tensor_add(o_sb[:, sl], o_sb[:, sl],
                                             bvb_sb[:, sl])
                    nc.sync.dma_start(out=out[t * P:(t + 1) * P, :], in_=o_sb)

    nc.finalize()
    return nc


def _prep_inputs(x, wq, bq, wk, bk, wv, bv):
    bf = ml_dtypes.bfloat16
    wq32 = np.asarray(wq, np.float32)
    wk32 = np.asarray(wk, np.float32)
    m_host = (wq32 @ wk32.T).astype(bf)                 # Wq Wk^T
    u_host = (wk32 @ np.asarray(bq, np.float32))        # Wk bq, [D]
    um = np.ascontiguousarray(u_host.reshape(8, P).T).astype(bf)
    wv_b = np.ascontiguousarray(wv, np.float32).astype(bf)
    bvr = np.asarray(bv, np.float32).reshape(1, D).astype(bf)

    i = np.arange(P)[:, None]
    j = np.arange(256)[None, :]
    masks = [np.where(j <= i + P * h, 0.0, NEG).astype(np.float32)
             for h in range(2)]

    in_maps = []
    for core in range(8):
        b, h = core // 2, core % 2
        xb = np.asarray(x[b], np.float32)
        xT = np.ascontiguousarray(xb.T).astype(bf)
        xR = xb.astype(bf)
        qcols = (np.arange(8)[:, None] * 2 + h) * P + np.arange(P)[None, :]
        xTq = np.ascontiguousarray(xT[:, qcols.ravel()])
        in_maps.append({
            "xt": xT, "xr": xR, "xtq": xTq, "mm_w": m_host, "wv": wv_b,
            "um": um, "bvr": bvr, "mask": masks[h],
        })
    return in_maps


def kernel(x, wq, bq, wk, bk, wv, bv, _trace=False, _trace_kwargs=None):
    if "nc" not in _CACHED:
        _CACHED["nc"] = build_nc()
    nc = _CACHED["nc"]
    in_maps = _prep_inputs(x, wq, bq, wk, bk, wv, bv)
    kw = {}
    if _trace:
        kw = dict(trace=True, **(_trace_kwargs or {}))
    res = run_bass_kernel_spmd(nc, in_maps, list(range(8)), **kw)
    out = np.empty((B, L, D), np.float32)
    for core in range(8):
        b, h = core // 2, core % 2
        o = np.asarray(res.results[core]["out"], np.float32)
        out[b].reshape(16, P, D)[h::2] = o.reshape(NQT, P, D)
    if _trace:
        _CACHED["last_results"] = res
    return out
